# revision 1
# baseline (speedup 1.0000x reference)
"""Trainium2 Bass kernel for nn_Attention (B=2,T=8,N=512,C=768,H=12).

Strategy: data-parallel over the 16 (b,t) slices -> 2 slices per core, 8 cores.
All weight/mask transposes are done on host. On-chip per slice:
  xT = transpose(x)                      (PE transposes, 128x128 tiles)
  qkvT[q,k] = W_qk @ xT                  ([d, n] layout; scale folded into Wq on host)
  v = xT.T @ W_v                         ([token, d] layout)
  ST[m,n] = kT.T @ qT ; P = exp(ST + maskT)   (no max-subtraction: logits bounded)
  OT'[0:64] = v_h.T @ P (PV), OT'[64] = row-sums l (ones column in v tiles)
  outT[c,n] = OT' * broadcast(1/l)       (ones-matmul broadcast of recip row)
  y = outT.T @ proj_wT                   ([n, C] layout, DMA out)
All matmuls run in float32r (full PE rate at N>=256).
"""
import sys

sys.path.insert(0, "/opt/trn_rl_repo")

import numpy as np
import concourse.bacc as bacc
import concourse.mybir as mybir
import concourse.tile as tile
from concourse.bass_utils import run_bass_kernel_spmd
from concourse.masks import make_identity

B, T, N, C = 2, 8, 512, 768
H = 12
Dh = C // H            # 64
SL = 2                 # slices per core
NCORES = 8
NC4 = N // 128         # 4 n-chunks
CC6 = C // 128         # 6 c-chunks
F32 = mybir.dt.float32
F32R = mybir.dt.float32r

_cache = {}


def build_nc():
    nc = bacc.Bacc()
    xs = nc.dram_tensor("xs", [SL, N, C], F32R, kind="ExternalInput")
    qkv_wTqk = nc.dram_tensor("qkv_wTqk", [C, 2 * C], F32R, kind="ExternalInput")
    qkv_wTv = nc.dram_tensor("qkv_wTv", [C, C], F32R, kind="ExternalInput")
    proj_wT = nc.dram_tensor("proj_wT", [C, C], F32R, kind="ExternalInput")
    maskT = nc.dram_tensor("maskT", [N, N], F32R, kind="ExternalInput")
    y = nc.dram_tensor("y", [SL, N, C], F32, kind="ExternalOutput")

    with tile.TileContext(nc) as tc:
        with (
            tc.tile_pool(name="wpool", bufs=1) as wpool,
            tc.tile_pool(name="sb", bufs=1) as sb,
            tc.tile_pool(name="ps", bufs=1, space="PSUM") as ps,
        ):
            # ---- persistent weights ----
            qkw = [wpool.tile([128, 2 * C], F32R, tag=f"qkw{cc}", name=f"qkw{cc}") for cc in range(CC6)]
            vw = [wpool.tile([128, C], F32R, tag=f"vw{cc}", name=f"vw{cc}") for cc in range(CC6)]
            projw = [wpool.tile([128, C], F32R, tag=f"projw{cc}", name=f"projw{cc}") for cc in range(CC6)]
            maskt = [wpool.tile([128, N], F32R, tag=f"maskt{mc}", name=f"maskt{mc}") for mc in range(NC4)]
            def emit_weight_dmas():
                for cc in range(CC6):
                    eng = nc.gpsimd if cc % 2 == 0 else nc.scalar
                    eng.dma_start(vw[cc][:], qkv_wTv[128 * cc:128 * (cc + 1), :])
                for cc in range(CC6):
                    eng = (nc.gpsimd, nc.scalar)[cc % 2]
                    eng.dma_start(qkw[cc][:], qkv_wTqk[128 * cc:128 * (cc + 1), :])
                for mc in range(NC4):
                    nc.sync.dma_start(maskt[mc][:], maskT[128 * mc:128 * (mc + 1), :])

            def emit_projw_dmas():
                for cc in range(CC6):
                    nc.gpsimd.dma_start(projw[cc][:], proj_wT[128 * cc:128 * (cc + 1), :])
            identf = wpool.tile([128, 128], F32, tag="identf")
            make_identity(nc, identf[:])
            ident = wpool.tile([128, 128], F32R, tag="ident")
            nc.vector.tensor_copy(ident[:], identf[:])
            onesf = wpool.tile([128, Dh], F32, tag="onesf")
            nc.gpsimd.memset(onesf[:], 1.0)

            xTs = [[None] * CC6 for _ in range(SL)]
            vsbs = [[None] * NC4 for _ in range(SL)]
            qks = [[None] * (2 * CC6) for _ in range(SL)]
            outTs = [[None] * CC6 for _ in range(SL)]

            def get(lst, i, mk):
                if lst[i] is None:
                    lst[i] = mk()
                return lst[i]

            def emit_transpose(s, n4):
                # one contiguous block DMA, then transpose 6 column chunks
                xblk = sb.tile([128, C], F32R, tag="xin", name=f"xblk{s}_{n4}", bufs=3)
                if s == 0 and n4 == 0:
                    # split the very first block so transposes start earlier
                    nc.sync.dma_start(xblk[:, 0:384], xs[s, 0:128, 0:384])
                    nc.sync.dma_start(xblk[:, 384:C], xs[s, 0:128, 384:C])
                else:
                    nc.sync.dma_start(xblk[:], xs[s, 128 * n4:128 * (n4 + 1), :])
                for cc in range(CC6):
                    xT = get(xTs[s], cc, lambda cc=cc: sb.tile(
                        [128, N], F32R, tag="xT", name=f"xT_s{s}_{cc}", bufs=8))
                    pt = ps.tile([128, 128], F32R, tag="ps1", name=f"pt{s}_{n4}_{cc}", bufs=8)
                    nc.tensor.transpose(pt[:], xblk[:, 128 * cc:128 * (cc + 1)], ident[:])
                    nc.vector.tensor_copy(xT[:, 128 * n4:128 * (n4 + 1)], pt[:])

            def emit_v(s, n4):
                xT = xTs[s]
                vsb = get(vsbs[s], n4, lambda: sb.tile(
                    [128, H * (Dh + 1)], F32R, tag="vsb", name=f"vsb_s{s}_{n4}", bufs=8))
                pva = ps.tile([128, 512], F32, tag="ps1", name=f"pva{s}_{n4}", bufs=8)
                pvb = ps.tile([128, 256], F32, tag="ps1", name=f"pvb{s}_{n4}", bufs=8)
                for i in range(CC6):
                    cc = (n4 + i) % CC6
                    lhsT = xT[cc][:, 128 * n4:128 * (n4 + 1)]
                    nc.tensor.matmul(pva[:], lhsT, vw[cc][:, 0:512],
                                     start=(i == 0), stop=(i == CC6 - 1))
                    nc.tensor.matmul(pvb[:], lhsT, vw[cc][:, 512:768],
                                     start=(i == 0), stop=(i == CC6 - 1))
                v3 = vsb[:].rearrange("p (h e) -> p h e", e=Dh + 1)
                cpy = nc.scalar.copy if s == 0 else nc.vector.tensor_copy
                cpy(v3[:, 0:8, 0:Dh], pva[:].rearrange("p (h e) -> p h e", e=Dh))
                cpy(v3[:, 8:12, 0:Dh], pvb[:].rearrange("p (h e) -> p h e", e=Dh))
                nc.vector.tensor_copy(v3[:, :, Dh:Dh + 1],
                                      onesf[:, 0:H].rearrange("p (h e) -> p h e", e=1))

            def emit_qk(s, jc):
                xT = xTs[s]
                qkt = get(qks[s], jc, lambda: sb.tile(
                    [128, N], F32R, tag="qk", name=f"qk_s{s}_{jc}", bufs=13))
                pqk = ps.tile([128, N], F32, tag="ps1", name=f"pqk{s}_{jc}", bufs=8)
                for i in range(CC6):
                    cc = (jc + i) % CC6
                    nc.tensor.matmul(pqk[:], qkw[cc][:, 128 * jc:128 * (jc + 1)], xT[cc][:],
                                     start=(i == 0), stop=(i == CC6 - 1))
                nc.vector.tensor_copy(qkt[:], pqk[:])

            def emit_head(s, h):
                qk, vsb = qks[s], vsbs[s]
                hb = 64 * (h % 2)
                qTh = qk[h // 2][hb:hb + 64, :]
                kTh = qk[CC6 + h // 2][hb:hb + 64, :]
                pts = []
                for mc in range(NC4):
                    pst = ps.tile([128, N], F32, tag="ps1", name=f"pst{s}_{h}_{mc}", bufs=8)
                    ptile = sb.tile([128, N], F32R, tag="pt", name=f"ptile{s}_{h}_{mc}", bufs=6)
                    if mc >= 2:
                        # mask added in-PSUM on DVE (PE/DVE load balance)
                        nc.tensor.matmul(pst[:], kTh[:, 128 * mc:128 * (mc + 1)], qTh,
                                         start=True, stop=True)
                        nc.vector.tensor_add(pst[:], pst[:], maskt[mc][:])
                    else:
                        # preload mask into PSUM (sets has_written), scores accumulate
                        nc.tensor.matmul(pst[:], ident[:], maskt[mc][:],
                                         start=True, stop=False, skip_group_check=True)
                        nc.tensor.matmul(pst[:], kTh[:, 128 * mc:128 * (mc + 1)], qTh,
                                         start=False, stop=True, skip_group_check=True)
                    nc.scalar.activation(ptile[:], pst[:],
                                         mybir.ActivationFunctionType.Exp)
                    pts.append(ptile)
                pot = ps.tile([Dh + 1, N], F32, tag="ps1", name=f"pot{s}_{h}", bufs=8)
                for mc in range(NC4):
                    nc.tensor.matmul(pot[:], vsb[mc][:, (Dh + 1) * h:(Dh + 1) * (h + 1)],
                                     pts[mc][:], start=(mc == 0), stop=(mc == NC4 - 1))
                recip = sb.tile([1, N], F32, tag="recip", name=f"recip{s}_{h}", bufs=3)
                nc.vector.reciprocal(recip[:], pot[Dh:Dh + 1, :])
                pbs = sb.tile([Dh, N], F32, tag="pbs", name=f"pbs{s}_{h}", bufs=3)
                nc.gpsimd.partition_broadcast(pbs[:], recip[:], channels=Dh)
                outT = get(outTs[s], h // 2, lambda: sb.tile(
                    [128, N], F32R, tag="outT", name=f"outT_s{s}_{h // 2}", bufs=10))
                with nc.allow_low_precision(reason="f32r outT"):
                    nc.vector.tensor_mul(outT[hb:hb + 64, :], pot[0:Dh, :], pbs[:])

            def emit_proj(s, n4):
                outT = outTs[s]
                if s == 1 and n4 == NC4 - 1:
                    # final unit: 3 narrow psum groups so the drain pipelines
                    osb = sb.tile([128, C], F32, tag="osb", name=f"osb{s}_{n4}", bufs=2)
                    for half in range(3):
                        c0 = 256 * half
                        pr = ps.tile([128, 256], F32, tag="ps1", name=f"pr{s}_{n4}_{half}", bufs=8)
                        for cc in range(CC6):
                            lhsT = outT[cc][:, 128 * n4:128 * (n4 + 1)]
                            nc.tensor.matmul(pr[:], lhsT, projw[cc][:, c0:c0 + 256],
                                             start=(cc == 0), stop=(cc == CC6 - 1))
                        eng = (nc.vector.tensor_copy, nc.scalar.copy)[half % 2]
                        eng(osb[:, c0:c0 + 256], pr[:])
                        deng = (nc.sync, nc.scalar)[half % 2]
                        deng.dma_start(y[s, 128 * n4:128 * (n4 + 1), c0:c0 + 256],
                                       osb[:, c0:c0 + 256])
                    return
                pra = ps.tile([128, 512], F32, tag="ps1", name=f"pra{s}_{n4}", bufs=8)
                prb = ps.tile([128, 256], F32, tag="ps1", name=f"prb{s}_{n4}", bufs=8)
                for cc in range(CC6):
                    lhsT = outT[cc][:, 128 * n4:128 * (n4 + 1)]
                    nc.tensor.matmul(pra[:], lhsT, projw[cc][:, 0:512],
                                     start=(cc == 0), stop=(cc == CC6 - 1))
                    nc.tensor.matmul(prb[:], lhsT, projw[cc][:, 512:768],
                                     start=(cc == 0), stop=(cc == CC6 - 1))
                osb = sb.tile([128, C], F32, tag="osb", name=f"osb{s}_{n4}", bufs=2)
                nc.vector.tensor_copy(osb[:, 0:512], pra[:])
                nc.sync.dma_start(y[s, 128 * n4:128 * (n4 + 1), 0:512], osb[:, 0:512])
                nc.scalar.copy(osb[:, 512:768], prb[:])
                nc.scalar.dma_start(y[s, 128 * n4:128 * (n4 + 1), 512:768], osb[:, 512:768])

            # ---- interleaved schedule ----
            for n4 in range(NC4):
                emit_transpose(0, n4)
            emit_weight_dmas()
            for n4 in range(NC4):
                emit_v(0, n4)
            for jc in range(2 * CC6):
                emit_qk(0, jc)
            # slice 0 attention interleaved with slice 1 early work
            e1 = [(emit_transpose, 1, n4) for n4 in range(NC4)] + \
                 [(emit_v, 1, n4) for n4 in range(NC4)] + \
                 [(emit_qk, 1, jc) for jc in range(2 * CC6)]
            k = 0
            for h in range(H):
                emit_head(0, h)
                if h == 3:
                    emit_projw_dmas()
                tgt = (len(e1) * (h + 1)) // H
                while k < tgt:
                    f, a, b = e1[k]; f(a, b); k += 1
            # slice 1 attention; slice 0 proj folded into the first heads
            p0 = [(emit_proj, 0, n4) for n4 in range(NC4)]
            k = 0
            for h in range(H):
                emit_head(1, h)
                if h < len(p0):
                    f, a, b = p0[k]; f(a, b); k += 1
            for n4 in range(NC4):
                emit_proj(1, n4)

    nc.finalize()
    return nc


def kernel(x, mask, qkv_w, q_bias, v_bias, proj_w, proj_b, _trace=False, _trace_kwargs=None):
    x, mask, qkv_w, proj_w = (np.asarray(a) for a in (x, mask, qkv_w, proj_w))
    q_bias, v_bias, proj_b = (np.asarray(a) for a in (q_bias, v_bias, proj_b))
    scale = Dh ** -0.5
    qkv_wT = np.ascontiguousarray(qkv_w.T).astype(np.float32)
    qkv_wT[:, :C] *= scale
    qkv_wTqk = np.ascontiguousarray(qkv_wT[:, :2 * C])
    qkv_wTv = np.ascontiguousarray(qkv_wT[:, 2 * C:])
    # biases folded in host-side only if nonzero (spec: all zeros). Assert to be safe.
    assert not np.any(q_bias) and not np.any(v_bias) and not np.any(proj_b), \
        "nonzero biases not supported by this kernel build"
    proj_wT = np.ascontiguousarray(proj_w.T).astype(np.float32)
    maskT = np.ascontiguousarray(mask.reshape(N, N).T).astype(np.float32)
    xf = np.ascontiguousarray(x.reshape(B * T, N, C)).astype(np.float32)

    if "nc" not in _cache:
        _cache["nc"] = build_nc()
    nc = _cache["nc"]

    in_maps = []
    for c in range(NCORES):
        in_maps.append({
            "xs": xf[SL * c:SL * (c + 1)],
            "qkv_wTqk": qkv_wTqk,
            "qkv_wTv": qkv_wTv,
            "proj_wT": proj_wT,
            "maskT": maskT,
        })
    res = run_bass_kernel_spmd(
        nc, in_maps, core_ids=list(range(NCORES)),
        trace=_trace, **(_trace_kwargs or {}),
    )
    out = np.concatenate([res.results[c]["y"] for c in range(NCORES)], axis=0)
    out = out.reshape(B, T, N, C)
    if _trace:
        return out, res
    return out



# revision 5
# speedup vs baseline: 1.2149x; 1.2149x over previous
"""Trainium2 Bass kernel for nn_Attention (B=2,T=8,N=512,C=768,H=12).

Data-parallel over 16 (b,t) slices -> 2 slices per core, 8 cores.
Structure (per core, 2 slices):
  - xT pre-transposed on HOST, uploaded bf16 -> plain contiguous DMA loads
    (no PE transposes, no XBAR). Weight arrays load as ONE DMA each
    (HWDGE issue slots are the startup bottleneck: 632ns per issue).
  - P = exp(S) * exp(maskT): exp on Act (bf16), multiply on DVE (2x bf16).
  - One continuous 24-head software pipeline across both slices:
    qk pairs emitted one pair ahead of their heads; PV lags scores by 2
    heads; outT multiply (on Pool) lags by ~4. Slice-1 early work and
    slice-0 proj are paced as fillers inside the stream.
  - PE p-state warmup: dummy matmuls at t~0.3us so the 3us ramp to full
    clock completes before real GEMMs start.
  - proj tail split: last two proj units accumulate cc 0..4 early, take the
    final outT-pair contribution at the end; last unit drains in 3 narrow
    psum groups.
"""
import sys

sys.path.insert(0, "/opt/trn_rl_repo")

import numpy as np
import ml_dtypes
import concourse.bacc as bacc
import concourse.mybir as mybir
import concourse.tile as tile
from concourse.bass_utils import run_bass_kernel_spmd

B, T, N, C = 2, 8, 512, 768
H = 12
Dh = C // H            # 64
SL = 2                 # slices per core
NCORES = 8
NC4 = N // 128         # 4 n-chunks
CC6 = C // 128         # 6 c-chunks
F32 = mybir.dt.float32
F32R = mybir.dt.float32r
BF16 = mybir.dt.bfloat16

_cache = {}


def build_nc():
    nc = bacc.Bacc()
    # xsT: host-transposed x, [SL, C, N]
    xsT = nc.dram_tensor("xsT", [SL, C, N], BF16, kind="ExternalInput")
    # qkwP: pair-major qkv q/k weights: [C, pair, (q|k) 256]
    qkwP = nc.dram_tensor("qkwP", [C, CC6, 256], BF16, kind="ExternalInput")
    qkv_wTv = nc.dram_tensor("qkv_wTv", [C, C], BF16, kind="ExternalInput")
    proj_wT = nc.dram_tensor("proj_wT", [C, C], BF16, kind="ExternalInput")
    expmT = nc.dram_tensor("expmT", [N, N], BF16, kind="ExternalInput")
    y = nc.dram_tensor("y", [SL, N, C], F32, kind="ExternalOutput")

    with tile.TileContext(nc) as tc:
        with (
            tc.tile_pool(name="wpool", bufs=1) as wpool,
            tc.tile_pool(name="sb", bufs=1) as sb,
            tc.tile_pool(name="ps", bufs=1, space="PSUM") as ps,
        ):
            # ---- persistent weights (single tiles, chunk views) ----
            # qk weights pair-major: qkwp[p][:, 256*cc + 128*k_half]
            qkwp = [wpool.tile([128, CC6 * 256], BF16, tag=f"qkwp{p}",
                               name=f"qkwp{p}") for p in range(CC6)]
            vwall = wpool.tile([128, CC6 * C], BF16, tag="vwall")
            projwall = wpool.tile([128, CC6 * C], BF16, tag="projwall")
            expmall = wpool.tile([128, NC4 * N], BF16, tag="expmall")
            vw = [vwall[:, C * cc:C * (cc + 1)] for cc in range(CC6)]
            projw = [projwall[:, C * cc:C * (cc + 1)] for cc in range(CC6)]
            expm = [expmall[:, N * mc:N * (mc + 1)] for mc in range(NC4)]

            def qkw_chunk(jc, cc):
                # stationary [128, 128] for qk unit jc, contraction chunk cc
                p, kk = jc % CC6, jc // CC6
                return qkwp[p][:, 256 * cc + 128 * kk:256 * cc + 128 * (kk + 1)]
            onesf = wpool.tile([128, 256], BF16, tag="onesf")
            nc.gpsimd.memset(onesf[:], 1.0)

            # PE p-state warmup: dummy matmuls bridge t~0.4us to the first
            # real GEMM (~3.2us) so the 3us ramp to full clock finishes first.
            pdum = ps.tile([Dh, 256], F32, tag="pspot", name="pdum", bufs=3)
            for i in range(15):
                nc.tensor.matmul(pdum[:], onesf[:, 0:Dh], onesf[:, 0:256],
                                 start=True, stop=True)

            xTall = [None] * SL
            vsbs = [[None] * NC4 for _ in range(SL)]
            qks = [[None] * (2 * CC6) for _ in range(SL)]
            outTs = [[None] * CC6 for _ in range(SL)]
            pots = {}
            recips = {}
            pts_map = {}
            proj_ps = {}

            def get(lst, i, mk):
                if lst[i] is None:
                    lst[i] = mk()
                return lst[i]

            def xT(s, cc):
                return xTall[s][:, N * cc:N * (cc + 1)]

            def in3(dram2d, nch, rows):
                # view [nch*128, rows] dram as [128, nch, rows]
                return dram2d[:, :].rearrange("(c p) j -> p c j", p=128)

            def emit_xT_dma(s, cc0, cc1, eng):
                out = xTall[s][:, N * cc0:N * cc1].rearrange(
                    "p (c j) -> p c j", j=N)
                src = xsT[s, 128 * cc0:128 * cc1, :].rearrange(
                    "(c p) j -> p c j", p=128)
                eng.dma_start(out, src)

            def emit_qkwp_dma(p, eng):
                out = qkwp[p][:].rearrange("p_ (c j) -> p_ c j", j=256)
                src = qkwP[:, p, :].rearrange("(c p_) j -> p_ c j", p_=128)
                eng.dma_start(out, src)

            def emit_startup_dmas():
                # transfer order tuned so pva(0,0) starts ~3.8us and no unit
                # stalls: xT0 n-half0, vw-lo in 2 cc-groups, xT0 n-half1,
                # qkw pair0, expm, vw-hi, qkw pairs 1..5. All first-needed
                # issues go on sync; scalar's queue starts with the act
                # table load (1.3us) so only late loads go there.
                xTall[0] = sb.tile([128, CC6 * N], BF16, tag="xta0", name="xTall0")
                xv = xTall[0][:].rearrange("p (c j) -> p c j", j=N)
                src = xsT[0, :, :].rearrange("(c p) j -> p c j", p=128)
                nc.sync.dma_start(xv[:, :, 0:256], src[:, :, 0:256])
                for g in range(2):
                    cc0 = 3 * g
                    vlo = vwall[:].rearrange("p (c j) -> p c j", j=C)
                    nc.sync.dma_start(
                        vlo[:, cc0:cc0 + 3, 0:512],
                        in3(qkv_wTv, CC6, C)[:, cc0:cc0 + 3, 0:512])
                nc.sync.dma_start(xv[:, :, 256:512], src[:, :, 256:512])
                emit_qkwp_dma(0, nc.scalar)
                eout = expmall[:].rearrange("p (c j) -> p c j", j=N)
                nc.scalar.dma_start(eout, in3(expmT, NC4, N))
                vhi = vwall[:].rearrange("p (c j) -> p c j", j=C)[:, :, 512:768]
                nc.sync.dma_start(
                    vhi, in3(qkv_wTv, CC6, C)[:, :, 512:768])
                for p in range(1, CC6):
                    emit_qkwp_dma(p, nc.scalar if p % 2 else nc.sync)

            def emit_xT1_dma():
                xTall[1] = sb.tile([128, CC6 * N], BF16, tag="xta1", name="xTall1")
                emit_xT_dma(1, 0, 6, nc.sync)

            def emit_projw_dmas():
                pout = projwall[:].rearrange("p (c j) -> p c j", j=C)
                nc.gpsimd.dma_start(pout, in3(proj_wT, CC6, C))

            def emit_v(s, n4, part="ab"):
                # part "a": heads 0-7 (vw cols 0:512); "b": heads 8-11
                vsb = get(vsbs[s], n4, lambda: sb.tile(
                    [128, H * (Dh + 1)], BF16, tag="vsb", name=f"vsb_s{s}_{n4}", bufs=8))
                v3 = vsb[:].rearrange("p (h e) -> p h e", e=Dh + 1)
                if "a" in part:
                    pva = ps.tile([128, 512], F32, tag="ps1", name=f"pva{s}_{n4}", bufs=5)
                    for i in range(CC6):
                        cc = (n4 + i) % CC6
                        lhsT = xT(s, cc)[:, 128 * n4:128 * (n4 + 1)]
                        nc.tensor.matmul(pva[:], lhsT, vw[cc][:, 0:512],
                                         start=(i == 0), stop=(i == CC6 - 1))
                    with nc.allow_low_precision(reason="bf16 v tiles"):
                        nc.vector.tensor_copy(v3[:, 0:8, 0:Dh],
                                              pva[:].rearrange("p (h e) -> p h e", e=Dh))
                        # ones column for ALL heads lives here so PV of heads
                        # 0-7 doesn't depend on the "b" part
                        nc.vector.tensor_copy(v3[:, :, Dh:Dh + 1],
                                              onesf[:, 0:H].rearrange("p (h e) -> p h e", e=1))
                if "b" in part:
                    pvb = ps.tile([128, 256], F32, tag="ps1", name=f"pvb{s}_{n4}", bufs=5)
                    for i in range(CC6):
                        cc = (n4 + i) % CC6
                        lhsT = xT(s, cc)[:, 128 * n4:128 * (n4 + 1)]
                        nc.tensor.matmul(pvb[:], lhsT, vw[cc][:, 512:768],
                                         start=(i == 0), stop=(i == CC6 - 1))
                    with nc.allow_low_precision(reason="bf16 v tiles"):
                        nc.vector.tensor_copy(v3[:, 8:12, 0:Dh],
                                              pvb[:].rearrange("p (h e) -> p h e", e=Dh))

            def emit_qk(s, jc, copy_eng):
                qkt = get(qks[s], jc, lambda: sb.tile(
                    [128, N], BF16, tag="qk", name=f"qk_s{s}_{jc}", bufs=13))
                pqk = ps.tile([128, N], F32, tag="ps1", name=f"pqk{s}_{jc}", bufs=5)
                for i in range(CC6):
                    cc = (jc + i) % CC6
                    nc.tensor.matmul(pqk[:], qkw_chunk(jc, cc), xT(s, cc),
                                     start=(i == 0), stop=(i == CC6 - 1))
                with nc.allow_low_precision(reason="bf16 qk tiles"):
                    copy_eng(qkt[:], pqk[:])

            def emit_scores(s, h):
                qk = qks[s]
                hb = 64 * (h % 2)
                qTh = qk[h // 2][hb:hb + 64, :]
                kTh = qk[CC6 + h // 2][hb:hb + 64, :]
                pts = []
                for mc in range(NC4):
                    pst = ps.tile([128, N], F32, tag="ps1", name=f"pst{s}_{h}_{mc}", bufs=5)
                    nc.tensor.matmul(pst[:], kTh[:, 128 * mc:128 * (mc + 1)], qTh,
                                     start=True, stop=True)
                    et = sb.tile([128, N], BF16, tag="et", name=f"et{s}_{h}_{mc}", bufs=9)
                    nc.scalar.activation(et[:], pst[:],
                                         mybir.ActivationFunctionType.Exp)
                    ptile = sb.tile([128, N], BF16, tag="pt", name=f"ptile{s}_{h}_{mc}", bufs=12)
                    with nc.allow_low_precision(reason="bf16 P"):
                        nc.vector.tensor_mul(ptile[:], et[:], expm[mc][:])
                    pts.append(ptile)
                pts_map[(s, h)] = pts

            def emit_pv(s, h):
                vsb = vsbs[s]
                pts = pts_map.pop((s, h))
                pot = ps.tile([Dh + 1, N], F32, tag="pspot", name=f"pot{s}_{h}", bufs=3)
                for mc in range(NC4):
                    nc.tensor.matmul(pot[:], vsb[mc][:, (Dh + 1) * h:(Dh + 1) * (h + 1)],
                                     pts[mc][:], start=(mc == 0), stop=(mc == NC4 - 1))
                recip = sb.tile([1, N], F32, tag="recip", name=f"recip{s}_{h}", bufs=4)
                nc.vector.reciprocal(recip[:], pot[Dh:Dh + 1, :])
                pbs = sb.tile([Dh, N], F32, tag="pbs", name=f"pbs{s}_{h}", bufs=4)
                nc.gpsimd.partition_broadcast(pbs[:], recip[:], channels=Dh)
                pots[(s, h)] = pot
                recips[(s, h)] = pbs

            def emit_outT(s, h):
                pot = pots.pop((s, h))
                pbs = recips.pop((s, h))
                hb = 64 * (h % 2)
                outT = get(outTs[s], h // 2, lambda: sb.tile(
                    [128, N], BF16, tag="outT", name=f"outT_s{s}_{h // 2}", bufs=12))
                # must be DVE: GPSIMD cannot access PSUM (walrus birverifier)
                with nc.allow_low_precision(reason="bf16 outT"):
                    nc.vector.tensor_mul(outT[hb:hb + 64, :], pot[0:Dh, :], pbs[:])

            def emit_proj_part(s, n4, cc_list, close, tail=False):
                outT = outTs[s]
                if (s, n4) not in proj_ps:
                    pra = ps.tile([128, 512], F32, tag="ps1", name=f"pra{s}_{n4}", bufs=5)
                    prb = ps.tile([128, 256], F32, tag="ps1", name=f"prb{s}_{n4}", bufs=5)
                    proj_ps[(s, n4)] = (pra, prb, [0])
                pra, prb, st = proj_ps[(s, n4)]
                for cc in cc_list:
                    first = st[0] == 0
                    last = close and cc == cc_list[-1]
                    nc.tensor.matmul(pra[:], outT[cc][:, 128 * n4:128 * (n4 + 1)],
                                     projw[cc][:, 0:512],
                                     start=first, stop=last, skip_group_check=True)
                    nc.tensor.matmul(prb[:], outT[cc][:, 128 * n4:128 * (n4 + 1)],
                                     projw[cc][:, 512:768],
                                     start=first, stop=last, skip_group_check=True)
                    st[0] += 1
                if not close:
                    return
                del proj_ps[(s, n4)]
                osb = sb.tile([128, C], F32, tag="osb", name=f"osb{s}_{n4}", bufs=2)
                with nc.allow_low_precision(reason="f32 copy"):
                    nc.vector.tensor_copy(osb[:, 0:512], pra[:])
                nc.sync.dma_start(y[s, 128 * n4:128 * (n4 + 1), 0:512], osb[:, 0:512])
                nc.scalar.copy(osb[:, 512:768], prb[:])
                nc.scalar.dma_start(y[s, 128 * n4:128 * (n4 + 1), 512:768], osb[:, 512:768])

            def emit_proj(s, n4):
                emit_proj_part(s, n4, list(range(CC6)), True)

            def emit_proj_final(s, n4):
                # 3 narrow psum groups; copies and y-DMA issues spread across
                # engines so the last drain pipelines
                outT = outTs[s]
                osb = sb.tile([128, C], F32, tag="osb", name=f"osb{s}_{n4}", bufs=2)
                copy_engs = (nc.vector.tensor_copy, nc.scalar.copy,
                             nc.vector.tensor_copy)
                dma_engs = (nc.sync, nc.scalar, nc.gpsimd)
                for half in range(3):
                    c0 = 256 * half
                    pr = ps.tile([128, 256], F32, tag="ps1", name=f"pr{s}_{n4}_{half}", bufs=5)
                    for cc in range(CC6):
                        nc.tensor.matmul(pr[:], outT[cc][:, 128 * n4:128 * (n4 + 1)],
                                         projw[cc][:, c0:c0 + 256],
                                         start=(cc == 0), stop=(cc == CC6 - 1))
                    with nc.allow_low_precision(reason="f32 copy"):
                        copy_engs[half](osb[:, c0:c0 + 256], pr[:])
                    dma_engs[half].dma_start(y[s, 128 * n4:128 * (n4 + 1), c0:c0 + 256],
                                             osb[:, c0:c0 + 256])

            # ---- startup ----
            emit_startup_dmas()
            # pva groups only (need just vw cols 0:512); pvb groups become
            # fillers inside the head pipeline (PV of heads 8-11 is late)
            for n4 in range(NC4):
                emit_v(0, n4, "a")

            # ---- continuous 24-head pipeline, pairs lead heads by 1 ----
            fillers = {
                0: [emit_xT1_dma],
                1: [lambda: emit_v(0, 0, "b")],
                2: [lambda: emit_v(0, 1, "b")],
                3: [lambda: emit_v(1, 0)],
                4: [lambda: emit_v(0, 2, "b")],
                5: [lambda: emit_v(1, 1)],
                6: [lambda: emit_v(0, 3, "b")],
                7: [lambda: emit_v(1, 2)],
                9: [lambda: emit_v(1, 3)],
                11: [emit_projw_dmas],
                15: [lambda: emit_proj(0, 0)],
                17: [lambda: emit_proj(0, 1)],
                19: [lambda: emit_proj(0, 2)],
                21: [lambda: emit_proj(0, 3)],
            }
            pairs = [(s, p) for s in (0, 1) for p in range(CC6)]
            heads = [(s, h) for s in (0, 1) for h in range(H)]
            sc_q = []
            pv_q = []
            for gi in range(H * SL):
                if gi == 0:
                    s, p = pairs[0]
                    emit_qk(s, p, nc.vector.tensor_copy)
                    emit_qk(s, CC6 + p, nc.scalar.copy)
                if gi % 2 == 0:
                    # emit pair (gi//2 + 1): one pair ahead of its heads
                    pi = gi // 2 + 1
                    if pi < len(pairs):
                        s, p = pairs[pi]
                        cpy_k = nc.scalar.copy if p % 2 == 0 else nc.vector.tensor_copy
                        emit_qk(s, p, nc.vector.tensor_copy)
                        emit_qk(s, CC6 + p, cpy_k)
                while len(pv_q) > 1:
                    emit_outT(*pv_q.pop(0))
                while len(sc_q) > 1:
                    key = sc_q.pop(0)
                    emit_pv(*key)
                    pv_q.append(key)
                emit_scores(*heads[gi])
                sc_q.append(heads[gi])
                for f in fillers.get(gi, []):
                    f()

            # ---- drain + split proj tail ----
            # loop end state: sc_q = [s1 h10, s1 h11], pv_q = [s1 h8, s1 h9]
            emit_outT(*pv_q.pop(0))             # s1 h8
            emit_outT(*pv_q.pop(0))             # s1 h9 (completes outT pair 4)
            key = sc_q.pop(0)                   # s1 h10
            emit_pv(*key); pv_q.append(key)
            emit_proj_part(1, 0, [0, 1, 2, 3, 4], False)
            key = sc_q.pop(0)                   # s1 h11
            emit_pv(*key); pv_q.append(key)
            emit_proj_part(1, 1, [0, 1, 2, 3, 4], False)
            emit_outT(*pv_q.pop(0))             # s1 h10
            emit_outT(*pv_q.pop(0))             # s1 h11 (completes outT pair 5)
            emit_proj_part(1, 0, [5], True)
            emit_proj_part(1, 1, [5], True)
            emit_proj_part(1, 2, list(range(CC6)), True, tail=True)
            emit_proj_final(1, 3)

    nc.finalize()
    return nc


def kernel(x, mask, qkv_w, q_bias, v_bias, proj_w, proj_b, _trace=False, _trace_kwargs=None):
    x, mask, qkv_w, proj_w = (np.asarray(a) for a in (x, mask, qkv_w, proj_w))
    q_bias, v_bias, proj_b = (np.asarray(a) for a in (q_bias, v_bias, proj_b))
    scale = Dh ** -0.5
    qkv_wT = np.ascontiguousarray(qkv_w.T).astype(np.float32)
    qkv_wT[:, :C] *= scale
    # pair-major q/k weights: qkwP[c, p, 0:128]=Wq row block jc=p,
    # qkwP[c, p, 128:256]=Wk row block jc=6+p
    wqk3 = qkv_wT[:, :2 * C].reshape(C, 2 * CC6, 128)
    qkwP = np.ascontiguousarray(
        np.stack([wqk3[:, [p, CC6 + p], :].reshape(C, 256) for p in range(CC6)],
                 axis=1)).astype(ml_dtypes.bfloat16)
    qkv_wTv = np.ascontiguousarray(qkv_wT[:, 2 * C:]).astype(ml_dtypes.bfloat16)
    # biases folded in host-side only if nonzero (spec: all zeros). Assert to be safe.
    assert not np.any(q_bias) and not np.any(v_bias) and not np.any(proj_b), \
        "nonzero biases not supported by this kernel build"
    proj_wT = np.ascontiguousarray(proj_w.T).astype(ml_dtypes.bfloat16)
    maskT = np.ascontiguousarray(mask.reshape(N, N).T).astype(np.float32)
    expmT = np.exp(maskT).astype(ml_dtypes.bfloat16)
    # host-side transpose: xsT[s] = x[s].T  (C, N)
    xsT = np.ascontiguousarray(
        x.reshape(B * T, N, C).transpose(0, 2, 1)).astype(ml_dtypes.bfloat16)

    if "nc" not in _cache:
        _cache["nc"] = build_nc()
    nc = _cache["nc"]

    in_maps = []
    for c in range(NCORES):
        in_maps.append({
            "xsT": xsT[SL * c:SL * (c + 1)],
            "qkwP": qkwP,
            "qkv_wTv": qkv_wTv,
            "proj_wT": proj_wT,
            "expmT": expmT,
        })
    res = run_bass_kernel_spmd(
        nc, in_maps, core_ids=list(range(NCORES)),
        trace=_trace, **(_trace_kwargs or {}),
    )
    out = np.concatenate([res.results[c]["y"] for c in range(NCORES)], axis=0)
    out = out.reshape(B, T, N, C)
    if _trace:
        return out, res
    return out


# revision 6
# speedup vs baseline: 1.2204x; 1.0046x over previous
"""Trainium2 Bass kernel for nn_Attention (B=2,T=8,N=512,C=768,H=12).

Data-parallel over 16 (b,t) slices -> 2 slices per core, 8 cores.
Structure (per core, 2 slices):
  - xT pre-transposed on HOST, uploaded bf16 -> plain contiguous DMA loads
    (no PE transposes, no XBAR). Weight arrays load as ONE DMA each
    (HWDGE issue slots are the startup bottleneck: 632ns per issue).
  - P = exp(S) * exp(maskT): exp on Act (bf16), multiply on DVE (2x bf16).
  - One continuous 24-head software pipeline across both slices:
    qk pairs emitted one pair ahead of their heads; PV lags scores by 2
    heads; outT multiply (on Pool) lags by ~4. Slice-1 early work and
    slice-0 proj are paced as fillers inside the stream.
  - PE p-state warmup: dummy matmuls at t~0.3us so the 3us ramp to full
    clock completes before real GEMMs start.
  - proj tail split: last two proj units accumulate cc 0..4 early, take the
    final outT-pair contribution at the end; last unit drains in 3 narrow
    psum groups.
"""
import sys

sys.path.insert(0, "/opt/trn_rl_repo")

import numpy as np
import ml_dtypes
import concourse.bacc as bacc
import concourse.mybir as mybir
import concourse.tile as tile
from concourse.bass_utils import run_bass_kernel_spmd

B, T, N, C = 2, 8, 512, 768
H = 12
Dh = C // H            # 64
SL = 2                 # slices per core
NCORES = 8
NC4 = N // 128         # 4 n-chunks
CC6 = C // 128         # 6 c-chunks
F32 = mybir.dt.float32
F32R = mybir.dt.float32r
BF16 = mybir.dt.bfloat16

_cache = {}


def build_nc():
    nc = bacc.Bacc()
    # xsT: host-transposed x, [SL, C, N]
    xsT = nc.dram_tensor("xsT", [SL, C, N], BF16, kind="ExternalInput")
    # qkwP: pair-major qkv q/k weights: [C, pair, (q|k) 256]
    qkwP = nc.dram_tensor("qkwP", [C, CC6, 256], BF16, kind="ExternalInput")
    qkv_wTv = nc.dram_tensor("qkv_wTv", [C, C], BF16, kind="ExternalInput")
    proj_wT = nc.dram_tensor("proj_wT", [C, C], BF16, kind="ExternalInput")
    expmT = nc.dram_tensor("expmT", [N, N], BF16, kind="ExternalInput")
    y = nc.dram_tensor("y", [SL, N, C], F32, kind="ExternalOutput")

    with tile.TileContext(nc) as tc:
        with (
            tc.tile_pool(name="wpool", bufs=1) as wpool,
            tc.tile_pool(name="sb", bufs=1) as sb,
            tc.tile_pool(name="ps", bufs=1, space="PSUM") as ps,
        ):
            # ---- persistent weights (single tiles, chunk views) ----
            # qk weights pair-major: qkwp[p][:, 256*cc + 128*k_half]
            qkwp = [wpool.tile([128, CC6 * 256], BF16, tag=f"qkwp{p}",
                               name=f"qkwp{p}") for p in range(CC6)]
            vwall = wpool.tile([128, CC6 * C], BF16, tag="vwall")
            projwall = wpool.tile([128, CC6 * C], BF16, tag="projwall")
            expmall = wpool.tile([128, NC4 * N], BF16, tag="expmall")
            vw = [vwall[:, C * cc:C * (cc + 1)] for cc in range(CC6)]
            projw = [projwall[:, C * cc:C * (cc + 1)] for cc in range(CC6)]
            expm = [expmall[:, N * mc:N * (mc + 1)] for mc in range(NC4)]

            def qkw_chunk(jc, cc):
                # stationary [128, 128] for qk unit jc, contraction chunk cc
                p, kk = jc % CC6, jc // CC6
                return qkwp[p][:, 256 * cc + 128 * kk:256 * cc + 128 * (kk + 1)]
            onesf = wpool.tile([128, 256], BF16, tag="onesf")
            nc.gpsimd.memset(onesf[:], 1.0)

            # PE p-state warmup: dummy matmuls bridge t~0.4us to the first
            # real GEMM (~3.2us) so the 3us ramp to full clock finishes first.
            pdum = ps.tile([Dh, 256], F32, tag="pspot", name="pdum", bufs=3)
            for i in range(15):
                nc.tensor.matmul(pdum[:], onesf[:, 0:Dh], onesf[:, 0:256],
                                 start=True, stop=True)

            xTall = [None] * SL
            vsbs = [[None] * NC4 for _ in range(SL)]
            qks = [[None] * (2 * CC6) for _ in range(SL)]
            outTs = [[None] * CC6 for _ in range(SL)]
            pots = {}
            recips = {}
            pts_map = {}
            proj_ps = {}

            def get(lst, i, mk):
                if lst[i] is None:
                    lst[i] = mk()
                return lst[i]

            def xT(s, cc):
                return xTall[s][:, N * cc:N * (cc + 1)]

            def in3(dram2d, nch, rows):
                # view [nch*128, rows] dram as [128, nch, rows]
                return dram2d[:, :].rearrange("(c p) j -> p c j", p=128)

            def emit_xT_dma(s, cc0, cc1, eng):
                out = xTall[s][:, N * cc0:N * cc1].rearrange(
                    "p (c j) -> p c j", j=N)
                src = xsT[s, 128 * cc0:128 * cc1, :].rearrange(
                    "(c p) j -> p c j", p=128)
                eng.dma_start(out, src)

            def emit_qkwp_dma(p, eng):
                out = qkwp[p][:].rearrange("p_ (c j) -> p_ c j", j=256)
                src = qkwP[:, p, :].rearrange("(c p_) j -> p_ c j", p_=128)
                eng.dma_start(out, src)

            def emit_startup_dmas():
                # transfer order tuned so pva(0,0) starts ~3.8us and no unit
                # stalls: xT0 n-half0, vw-lo in 2 cc-groups, xT0 n-half1,
                # qkw pair0, expm, vw-hi, qkw pairs 1..5. All first-needed
                # issues go on sync; scalar's queue starts with the act
                # table load (1.3us) so only late loads go there.
                xTall[0] = sb.tile([128, CC6 * N], BF16, tag="xta0", name="xTall0")
                xv = xTall[0][:].rearrange("p (c j) -> p c j", j=N)
                src = xsT[0, :, :].rearrange("(c p) j -> p c j", p=128)
                nc.sync.dma_start(xv[:, :, 0:256], src[:, :, 0:256])
                for g in range(2):
                    cc0 = 3 * g
                    vlo = vwall[:].rearrange("p (c j) -> p c j", j=C)
                    nc.sync.dma_start(
                        vlo[:, cc0:cc0 + 3, 0:512],
                        in3(qkv_wTv, CC6, C)[:, cc0:cc0 + 3, 0:512])
                nc.sync.dma_start(xv[:, :, 256:512], src[:, :, 256:512])
                emit_qkwp_dma(0, nc.scalar)
                eout = expmall[:].rearrange("p (c j) -> p c j", j=N)
                nc.scalar.dma_start(eout, in3(expmT, NC4, N))
                vhi = vwall[:].rearrange("p (c j) -> p c j", j=C)[:, :, 512:768]
                nc.sync.dma_start(
                    vhi, in3(qkv_wTv, CC6, C)[:, :, 512:768])
                for p in range(1, CC6):
                    emit_qkwp_dma(p, nc.scalar if p % 2 else nc.sync)

            def emit_xT1_dma():
                xTall[1] = sb.tile([128, CC6 * N], BF16, tag="xta1", name="xTall1")
                emit_xT_dma(1, 0, 6, nc.sync)

            def emit_projw_dmas():
                pout = projwall[:].rearrange("p (c j) -> p c j", j=C)
                nc.gpsimd.dma_start(pout, in3(proj_wT, CC6, C))

            def emit_v(s, n4, part="ab"):
                # part "a": heads 0-7 (vw cols 0:512); "b": heads 8-11
                vsb = get(vsbs[s], n4, lambda: sb.tile(
                    [128, H * (Dh + 1)], BF16, tag="vsb", name=f"vsb_s{s}_{n4}", bufs=8))
                v3 = vsb[:].rearrange("p (h e) -> p h e", e=Dh + 1)
                if "a" in part:
                    pva = ps.tile([128, 512], F32, tag="ps1", name=f"pva{s}_{n4}", bufs=5)
                    for i in range(CC6):
                        cc = (n4 + i) % CC6
                        lhsT = xT(s, cc)[:, 128 * n4:128 * (n4 + 1)]
                        nc.tensor.matmul(pva[:], lhsT, vw[cc][:, 0:512],
                                         start=(i == 0), stop=(i == CC6 - 1))
                    with nc.allow_low_precision(reason="bf16 v tiles"):
                        nc.vector.tensor_copy(v3[:, 0:8, 0:Dh],
                                              pva[:].rearrange("p (h e) -> p h e", e=Dh))
                        # ones column for ALL heads lives here so PV of heads
                        # 0-7 doesn't depend on the "b" part
                        nc.vector.tensor_copy(v3[:, :, Dh:Dh + 1],
                                              onesf[:, 0:H].rearrange("p (h e) -> p h e", e=1))
                if "b" in part:
                    pvb = ps.tile([128, 256], F32, tag="ps1", name=f"pvb{s}_{n4}", bufs=5)
                    for i in range(CC6):
                        cc = (n4 + i) % CC6
                        lhsT = xT(s, cc)[:, 128 * n4:128 * (n4 + 1)]
                        nc.tensor.matmul(pvb[:], lhsT, vw[cc][:, 512:768],
                                         start=(i == 0), stop=(i == CC6 - 1))
                    with nc.allow_low_precision(reason="bf16 v tiles"):
                        nc.vector.tensor_copy(v3[:, 8:12, 0:Dh],
                                              pvb[:].rearrange("p (h e) -> p h e", e=Dh))

            def emit_qk(s, jc, copy_eng):
                qkt = get(qks[s], jc, lambda: sb.tile(
                    [128, N], BF16, tag="qk", name=f"qk_s{s}_{jc}", bufs=13))
                pqk = ps.tile([128, N], F32, tag="ps1", name=f"pqk{s}_{jc}", bufs=5)
                for i in range(CC6):
                    cc = (jc + i) % CC6
                    nc.tensor.matmul(pqk[:], qkw_chunk(jc, cc), xT(s, cc),
                                     start=(i == 0), stop=(i == CC6 - 1))
                with nc.allow_low_precision(reason="bf16 qk tiles"):
                    copy_eng(qkt[:], pqk[:])

            def emit_scores(s, h):
                qk = qks[s]
                hb = 64 * (h % 2)
                qTh = qk[h // 2][hb:hb + 64, :]
                kTh = qk[CC6 + h // 2][hb:hb + 64, :]
                pts = []
                for mc in range(NC4):
                    pst = ps.tile([128, N], F32, tag="ps1", name=f"pst{s}_{h}_{mc}", bufs=5)
                    nc.tensor.matmul(pst[:], kTh[:, 128 * mc:128 * (mc + 1)], qTh,
                                     start=True, stop=True)
                    et = sb.tile([128, N], BF16, tag="et", name=f"et{s}_{h}_{mc}", bufs=9)
                    nc.scalar.activation(et[:], pst[:],
                                         mybir.ActivationFunctionType.Exp)
                    ptile = sb.tile([128, N], BF16, tag="pt", name=f"ptile{s}_{h}_{mc}", bufs=12)
                    # all-SBUF bf16 multiply; mc 3 runs on Pool to keep DVE
                    # under the PE cycle (GPSIMD may not touch PSUM, but this
                    # one is SBUF-only)
                    eng_mul = nc.gpsimd.tensor_mul if mc == 3 else nc.vector.tensor_mul
                    with nc.allow_low_precision(reason="bf16 P"):
                        eng_mul(ptile[:], et[:], expm[mc][:])
                    pts.append(ptile)
                pts_map[(s, h)] = pts

            def emit_pv(s, h):
                vsb = vsbs[s]
                pts = pts_map.pop((s, h))
                pot = ps.tile([Dh + 1, N], F32, tag="pspot", name=f"pot{s}_{h}", bufs=3)
                for mc in range(NC4):
                    nc.tensor.matmul(pot[:], vsb[mc][:, (Dh + 1) * h:(Dh + 1) * (h + 1)],
                                     pts[mc][:], start=(mc == 0), stop=(mc == NC4 - 1))
                recip = sb.tile([1, N], F32, tag="recip", name=f"recip{s}_{h}", bufs=4)
                nc.vector.reciprocal(recip[:], pot[Dh:Dh + 1, :])
                pbs = sb.tile([Dh, N], F32, tag="pbs", name=f"pbs{s}_{h}", bufs=4)
                nc.gpsimd.partition_broadcast(pbs[:], recip[:], channels=Dh)
                pots[(s, h)] = pot
                recips[(s, h)] = pbs

            def emit_outT(s, h):
                pot = pots.pop((s, h))
                pbs = recips.pop((s, h))
                hb = 64 * (h % 2)
                outT = get(outTs[s], h // 2, lambda: sb.tile(
                    [128, N], BF16, tag="outT", name=f"outT_s{s}_{h // 2}", bufs=12))
                # must be DVE: GPSIMD cannot access PSUM (walrus birverifier)
                with nc.allow_low_precision(reason="bf16 outT"):
                    nc.vector.tensor_mul(outT[hb:hb + 64, :], pot[0:Dh, :], pbs[:])

            def emit_proj_part(s, n4, cc_list, close, tail=False):
                outT = outTs[s]
                if (s, n4) not in proj_ps:
                    pra = ps.tile([128, 512], F32, tag="ps1", name=f"pra{s}_{n4}", bufs=5)
                    prb = ps.tile([128, 256], F32, tag="ps1", name=f"prb{s}_{n4}", bufs=5)
                    proj_ps[(s, n4)] = (pra, prb, [0])
                pra, prb, st = proj_ps[(s, n4)]
                for cc in cc_list:
                    first = st[0] == 0
                    last = close and cc == cc_list[-1]
                    nc.tensor.matmul(pra[:], outT[cc][:, 128 * n4:128 * (n4 + 1)],
                                     projw[cc][:, 0:512],
                                     start=first, stop=last, skip_group_check=True)
                    nc.tensor.matmul(prb[:], outT[cc][:, 128 * n4:128 * (n4 + 1)],
                                     projw[cc][:, 512:768],
                                     start=first, stop=last, skip_group_check=True)
                    st[0] += 1
                if not close:
                    return
                del proj_ps[(s, n4)]
                osb = sb.tile([128, C], F32, tag="osb", name=f"osb{s}_{n4}", bufs=2)
                with nc.allow_low_precision(reason="f32 copy"):
                    nc.vector.tensor_copy(osb[:, 0:512], pra[:])
                nc.sync.dma_start(y[s, 128 * n4:128 * (n4 + 1), 0:512], osb[:, 0:512])
                nc.scalar.copy(osb[:, 512:768], prb[:])
                nc.scalar.dma_start(y[s, 128 * n4:128 * (n4 + 1), 512:768], osb[:, 512:768])

            def emit_proj(s, n4):
                emit_proj_part(s, n4, list(range(CC6)), True)

            def emit_proj_final(s, n4):
                # 3 narrow psum groups; copies and y-DMA issues spread across
                # engines so the last drain pipelines
                outT = outTs[s]
                osb = sb.tile([128, C], F32, tag="osb", name=f"osb{s}_{n4}", bufs=2)
                copy_engs = (nc.vector.tensor_copy, nc.scalar.copy,
                             nc.vector.tensor_copy)
                dma_engs = (nc.sync, nc.scalar, nc.gpsimd)
                for half in range(3):
                    c0 = 256 * half
                    pr = ps.tile([128, 256], F32, tag="ps1", name=f"pr{s}_{n4}_{half}", bufs=5)
                    for cc in range(CC6):
                        nc.tensor.matmul(pr[:], outT[cc][:, 128 * n4:128 * (n4 + 1)],
                                         projw[cc][:, c0:c0 + 256],
                                         start=(cc == 0), stop=(cc == CC6 - 1))
                    with nc.allow_low_precision(reason="f32 copy"):
                        copy_engs[half](osb[:, c0:c0 + 256], pr[:])
                    dma_engs[half].dma_start(y[s, 128 * n4:128 * (n4 + 1), c0:c0 + 256],
                                             osb[:, c0:c0 + 256])

            # ---- startup ----
            emit_startup_dmas()
            # pva groups only (need just vw cols 0:512); pvb groups become
            # fillers inside the head pipeline (PV of heads 8-11 is late)
            for n4 in range(NC4):
                emit_v(0, n4, "a")

            # ---- continuous 24-head pipeline, pairs lead heads by 1 ----
            fillers = {
                0: [emit_xT1_dma],
                1: [lambda: emit_v(0, 0, "b")],
                2: [lambda: emit_v(0, 1, "b")],
                3: [lambda: emit_v(1, 0)],
                4: [lambda: emit_v(0, 2, "b")],
                5: [lambda: emit_v(1, 1)],
                6: [lambda: emit_v(0, 3, "b")],
                7: [lambda: emit_v(1, 2)],
                9: [lambda: emit_v(1, 3)],
                11: [emit_projw_dmas],
                15: [lambda: emit_proj(0, 0)],
                17: [lambda: emit_proj(0, 1)],
                19: [lambda: emit_proj(0, 2)],
                21: [lambda: emit_proj(0, 3)],
            }
            pairs = [(s, p) for s in (0, 1) for p in range(CC6)]
            heads = [(s, h) for s in (0, 1) for h in range(H)]
            sc_q = []
            pv_q = []
            for gi in range(H * SL):
                if gi == 0:
                    s, p = pairs[0]
                    emit_qk(s, p, nc.vector.tensor_copy)
                    emit_qk(s, CC6 + p, nc.scalar.copy)
                if gi % 2 == 0:
                    # emit pair (gi//2 + 1): one pair ahead of its heads
                    pi = gi // 2 + 1
                    if pi < len(pairs):
                        s, p = pairs[pi]
                        cpy_k = nc.scalar.copy if p % 2 == 0 else nc.vector.tensor_copy
                        emit_qk(s, p, nc.vector.tensor_copy)
                        emit_qk(s, CC6 + p, cpy_k)
                while len(pv_q) > 1:
                    emit_outT(*pv_q.pop(0))
                while len(sc_q) > 1:
                    key = sc_q.pop(0)
                    emit_pv(*key)
                    pv_q.append(key)
                emit_scores(*heads[gi])
                sc_q.append(heads[gi])
                for f in fillers.get(gi, []):
                    f()

            # ---- drain + split proj tail ----
            # loop end state: sc_q = [s1 h10, s1 h11], pv_q = [s1 h8, s1 h9]
            emit_outT(*pv_q.pop(0))             # s1 h8
            emit_outT(*pv_q.pop(0))             # s1 h9 (completes outT pair 4)
            key = sc_q.pop(0)                   # s1 h10
            emit_pv(*key); pv_q.append(key)
            emit_proj_part(1, 0, [0, 1, 2, 3, 4], False)
            key = sc_q.pop(0)                   # s1 h11
            emit_pv(*key); pv_q.append(key)
            emit_proj_part(1, 1, [0, 1, 2, 3, 4], False)
            emit_outT(*pv_q.pop(0))             # s1 h10
            emit_outT(*pv_q.pop(0))             # s1 h11 (completes outT pair 5)
            emit_proj_part(1, 0, [5], True)
            emit_proj_part(1, 1, [5], True)
            emit_proj_part(1, 2, list(range(CC6)), True, tail=True)
            emit_proj_final(1, 3)

    nc.finalize()
    return nc


def kernel(x, mask, qkv_w, q_bias, v_bias, proj_w, proj_b, _trace=False, _trace_kwargs=None):
    x, mask, qkv_w, proj_w = (np.asarray(a) for a in (x, mask, qkv_w, proj_w))
    q_bias, v_bias, proj_b = (np.asarray(a) for a in (q_bias, v_bias, proj_b))
    scale = Dh ** -0.5
    qkv_wT = np.ascontiguousarray(qkv_w.T).astype(np.float32)
    qkv_wT[:, :C] *= scale
    # pair-major q/k weights: qkwP[c, p, 0:128]=Wq row block jc=p,
    # qkwP[c, p, 128:256]=Wk row block jc=6+p
    wqk3 = qkv_wT[:, :2 * C].reshape(C, 2 * CC6, 128)
    qkwP = np.ascontiguousarray(
        np.stack([wqk3[:, [p, CC6 + p], :].reshape(C, 256) for p in range(CC6)],
                 axis=1)).astype(ml_dtypes.bfloat16)
    qkv_wTv = np.ascontiguousarray(qkv_wT[:, 2 * C:]).astype(ml_dtypes.bfloat16)
    # biases folded in host-side only if nonzero (spec: all zeros). Assert to be safe.
    assert not np.any(q_bias) and not np.any(v_bias) and not np.any(proj_b), \
        "nonzero biases not supported by this kernel build"
    proj_wT = np.ascontiguousarray(proj_w.T).astype(ml_dtypes.bfloat16)
    maskT = np.ascontiguousarray(mask.reshape(N, N).T).astype(np.float32)
    expmT = np.exp(maskT).astype(ml_dtypes.bfloat16)
    # host-side transpose: xsT[s] = x[s].T  (C, N)
    xsT = np.ascontiguousarray(
        x.reshape(B * T, N, C).transpose(0, 2, 1)).astype(ml_dtypes.bfloat16)

    if "nc" not in _cache:
        _cache["nc"] = build_nc()
    nc = _cache["nc"]

    in_maps = []
    for c in range(NCORES):
        in_maps.append({
            "xsT": xsT[SL * c:SL * (c + 1)],
            "qkwP": qkwP,
            "qkv_wTv": qkv_wTv,
            "proj_wT": proj_wT,
            "expmT": expmT,
        })
    res = run_bass_kernel_spmd(
        nc, in_maps, core_ids=list(range(NCORES)),
        trace=_trace, **(_trace_kwargs or {}),
    )
    out = np.concatenate([res.results[c]["y"] for c in range(NCORES)], axis=0)
    out = out.reshape(B, T, N, C)
    if _trace:
        return out, res
    return out
